# revision 73
# baseline (speedup 1.0000x reference)
"""Trainium2 Bass kernel for the soft-decision-tree ensemble problem.

Math (per reference):
  sel[e,n] = argmax_d T[e,n,:] ; t[e,n] = max_d T[e,n,:]
  s[b,en]  = floor(t[en] - x[b, sel[en]])
  p[b,e,l] = prod_j (bit ? 1-s : s) over the leaf's 6 ancestors
  out      = softmax(p @ L, axis=classes)

Strategy (v4.11, ~79us vs 151us baseline): batch-parallel across 8
cores, T/L replicated.
- x is shipped as a transposed fp16 hi/lo split plane xp[512, 2048]
  (row d = [hi[:,d] | lo[:,d]]); reconstruction error 2^-21 -> 3 floor
  flips in 8.2M on this dataset (end-to-end 1.7e-5).
- Feature selection runs on the DMA engines via dma_gather(transpose=
  True) in four 256-index quarters: each selected feature row (4KB) is
  transposed at u16 granularity straight into [b-partition, b-chunk,
  node-slot] layout. GPSIMD only generates descriptors (mlp library;
  a dep-free dummy gather preloads the ucode early).
- T is host-reordered (Tp) so each partition loads one contiguous run
  and tmax/argmax land directly in gather-slot order; sel_sb (16-part
  wrap, replicated x8) and t_bc are built on-chip with two PE
  permutation matmuls per quarter (repq/selm/onesel host constants) --
  no DRAM roundtrips anywhere in the prefix.
- s = rint((t - 0.5) - (hi + lo)): two DVE passes; the int16 rint cast
  is fused into the second subtract (HW DVE rounds to nearest).
- Tree with signed factors f0=s, f1'=s-1 per estimator-half, split in
  two 4-chunk groups so PE transposes overlap the tree. Slots are
  host-permuted (bit-reversed within each level) so every level writes
  plane-major with contiguous 2-byte operands (DVE 2x-eligible); the
  leaf permutation is folded into a host-permuted L. Leaf values are
  integers <= 8000 so fp16 carries them at 1.7e-5 end-to-end error.
- Final contraction is flipped: Lmod el-chunks are the stationary
  operand (8 LDWEIGHTS total) and vT streams 512-wide fp16 columns,
  accumulating y^T[100, 1024] in PSUM. y^T transposes back (fp32) and
  a vectorized softmax runs per 4-chunk group with early output DMA.
- (-1)^popcount(path) signs fold into Lmod via a host parity constant
  (invariant under the bit-reversal).
"""
import os
import sys

for p in ("/opt/trn_rl_repo",):
    if p not in sys.path and os.path.isdir(p):
        sys.path.insert(0, p)

import numpy as np

import concourse.bass as bass
import concourse.tile as tile
from concourse import bacc, mybir
from concourse.bass_utils import run_bass_kernel_spmd

# problem constants (hardcoded per contract)
B, D = 8192, 512
E, NN, NL, C = 16, 63, 64, 100
DEPTH = 6
NCORES = 8
BC = B // NCORES          # rows per core = 1024
CH = BC // 128            # 128-row chunks per core = 8
NNP = 64                  # padded nodes per estimator
ENP = E * NNP             # 1024 padded node slots
EH = ENP // 2             # 512 per estimator half
EHF = E // 2              # estimators per half = 8

F32 = mybir.dt.float32
F16 = mybir.dt.float16
I16 = mybir.dt.int16
I32 = mybir.dt.int32
AX = mybir.AxisListType
OP = mybir.AluOpType
AF = mybir.ActivationFunctionType

SCAST = os.environ.get("KERNEL_SCAST", "dve")  # act | dve


def build_program():
    nc = bacc.Bacc(
        "TRN2",
        target_bir_lowering=False,
        debug=False,
        enable_asserts=False,
        num_devices=NCORES,
    )

    xp_in = nc.dram_tensor("xp", [D, 2 * BC], F16, kind="ExternalInput").ap()
    T_in = nc.dram_tensor("Tp", [ENP, D], F32, kind="ExternalInput").ap()
    L_in = nc.dram_tensor("L", [E, NL, C], F32, kind="ExternalInput").ap()
    idf_in = nc.dram_tensor("idf", [128, 128], F16, kind="ExternalInput").ap()
    idf32_in = nc.dram_tensor("idf32", [128, 128], F32, kind="ExternalInput").ap()
    iota_in = nc.dram_tensor("iotab", [128, D], F32, kind="ExternalInput").ap()
    repq_in = nc.dram_tensor("repq", [128, 128], F16, kind="ExternalInput").ap()
    selm_in = nc.dram_tensor("selm", [128, 32], F16, kind="ExternalInput").ap()
    onesel_in = nc.dram_tensor("onesel", [2, 256], F32, kind="ExternalInput").ap()
    sgn_in = nc.dram_tensor("sgn", [128, 1], F32, kind="ExternalInput").ap()
    out_d = nc.dram_tensor("out", [128, CH, C], F32, kind="ExternalOutput").ap()
    dum_scr = nc.dram_tensor("dum_scr", [4, 256], F16).ap()

    with tile.TileContext(nc) as tc:
        with (
            tc.tile_pool(name="const", bufs=1) as constp,
            tc.tile_pool(name="tproc", bufs=1) as tprocp,
            tc.tile_pool(name="big", bufs=1) as bigp,
            tc.tile_pool(name="work", bufs=2) as workp,
            tc.tile_pool(name="acc", bufs=1) as accp,
            tc.tile_pool(name="psum1", bufs=1, space="PSUM") as psum1,
            tc.tile_pool(name="psum_tp", bufs=3, space="PSUM") as psumtp,
            tc.tile_pool(name="psum_y", bufs=1, space="PSUM") as psumy,
            tc.tile_pool(name="psum_sm", bufs=2, space="PSUM") as psumsm,
        ):
            # ---- T load first: Tp is host-reordered so every partition
            # reads one contiguous run (row r -> (p, t) = (r//8, r%8);
            # gather slot j = 512*(t//4) + 128*(t%4) + p).
            T_sb = tprocp.tile([128, 8, D], F32)
            T_v = T_in.rearrange("(p t) d -> p t d", t=8)
            nc.sync.dma_start(T_sb[:, 0:4, :], T_v[:, 0:4, :])
            nc.scalar.dma_start(T_sb[:, 4:8, :], T_v[:, 4:8, :])

            # ---- tiny constants ----
            sgn = constp.tile([128, 1], F32)
            nc.scalar.dma_start(sgn[:], sgn_in[:])
            iota = constp.tile([128, D], F32)
            nc.sync.dma_start(iota[:], iota_in[:])
            repq = constp.tile([128, 128], F16)
            nc.scalar.dma_start(repq[:], repq_in[:])
            selm = constp.tile([128, 32], F16)
            nc.sync.dma_start(selm[:], selm_in[:])
            onesel = constp.tile([2, 256], F32)
            nc.scalar.dma_start(onesel[:], onesel_in[:])
            ones = constp.tile([1, 128], F32)
            nc.vector.memset(ones[:], 1.0)

            # ---- dummy dma_gather: preloads the GPSIMD mlp ucode library
            # (descriptor generation) so the real gathers don't pay the
            # ~6us IRAM swap on the critical path. Gathers from xp (an
            # external input) so it has no device-side data deps and can
            # run immediately after the library load.
            dummy_idx = constp.tile([128, 8], I16)
            nc.vector.memset(dummy_idx[:], 0)
            dummy_out = constp.tile([128, 16, 128], F16)
            nc.gpsimd.dma_gather(
                dummy_out[:], xp_in[:], dummy_idx[:],
                num_idxs=128, num_idxs_reg=128, elem_size=2 * BC,
                transpose=True,
            )

            idf = constp.tile([128, 128], F16)
            nc.sync.dma_start(idf[:], idf_in[:])
            idf32 = constp.tile([128, 128], F32)
            nc.scalar.dma_start(idf32[:], idf32_in[:])

            # ---- T processing + on-chip sel/t redistribution, per half ----
            # slot j = 512h + 128*t + p ; q = j%16 = p%16 ; f = j//16 =
            # 8t + p//16.
            sel_sb = constp.tile([128, ENP // 16], I16)
            t_bc = constp.tile([128, ENP], F32)
            xg = [None, None]

            EQ = 256  # slots per quarter-gather
            xg = [None] * 4
            for k in range(4):
                ts0 = 2 * k  # global t-cols (ts0, ts0+1); h = k // 2
                tq = 2 * (k % 2)
                tmax = tprocp.tile([128, 2], F32, tag=f"tmax{k}", name=f"tmax{k}")
                sel_f = tprocp.tile([128, 2], F32, tag=f"self{k}", name=f"self{k}")
                nc.vector.tensor_reduce(
                    tmax[:], T_sb[:, ts0 : ts0 + 2, :], axis=AX.X, op=OP.max
                )
                for t in range(2):
                    scr = workp.tile([128, D], F32, tag="tscr")
                    nc.vector.scalar_tensor_tensor(
                        scr[:],
                        T_sb[:, ts0 + t, :],
                        tmax[:, t : t + 1],
                        iota[:, :],
                        op0=OP.is_equal,
                        op1=OP.mult,
                        accum_out=sel_f[:, t : t + 1],
                    )

                # sel_sb[q + 16g, 16k + 8t + a] = sel_f[16a + q, t]
                rhs_sel = tprocp.tile(
                    [128, 2, 8], F16, tag=f"rsel{k}", name=f"rsel{k}"
                )
                nc.vector.tensor_tensor(
                    rhs_sel[:],
                    sel_f[:].unsqueeze(2).broadcast_to([128, 2, 8]),
                    selm[:].rearrange("p (t a) -> p t a", a=8)[:, tq : tq + 2, :],
                    op=OP.mult,
                )
                sel_ps = psum1.tile([128, EQ], F32, tag="tbc")
                nc.tensor.matmul(
                    sel_ps[:, 0:16],
                    lhsT=repq[:],
                    rhs=rhs_sel[:].rearrange("p t a -> p (t a)"),
                    start=True,
                    stop=True,
                )
                nc.vector.tensor_copy(
                    sel_sb[:, k * 16 : (k + 1) * 16], sel_ps[:, 0:16]
                )

                # t_bc[:, 256k + 128t + p] = tmax[p, t] - 0.5
                tT_ps0 = psum1.tile([128, EQ], F32, tag="tbc")
                tT_ps = tT_ps0[:2, 0:128]
                nc.tensor.transpose(tT_ps, tmax[:], idf32[:])
                tmaxT = tprocp.tile([2, 128], F32, tag=f"tmT{k}", name=f"tmT{k}")
                nc.scalar.activation(tmaxT[:], tT_ps, AF.Copy)
                tb_ps = psum1.tile([128, EQ], F32, tag="tbc")
                for t in range(2):
                    nc.tensor.matmul(
                        tb_ps[:, t * 128 : (t + 1) * 128],
                        lhsT=onesel[:, t * 128 : (t + 1) * 128],
                        rhs=tmaxT[:],
                        start=True,
                        stop=True,
                    )
                nc.scalar.activation(
                    t_bc[:, k * EQ : (k + 1) * EQ], tb_ps[:], AF.Copy, bias=-0.5
                )

                # gather: xg[k][p, m, i] = xp[sel[256k+i], m*128 + p]
                xg[k] = bigp.tile(
                    [128, 16, EQ], F16, tag=f"xg{k}", name=f"xg{k}"
                )
                nc.gpsimd.dma_gather(
                    xg[k][:],
                    xp_in[:],
                    sel_sb[:, k * 16 : (k + 1) * 16],
                    num_idxs=EQ,
                    num_idxs_reg=EQ,
                    elem_size=2 * BC,
                    transpose=True,
                )

            # ---- Lmod loads + build (off the critical queues) ----
            Lpair = L_in.rearrange("e (m two) c -> (e m) (two c)", two=2)
            Lodd = Lpair[:, C : 2 * C].rearrange("(q p) c -> p q c", p=128)
            Leven = Lpair[:, 0:C].rearrange("(q p) c -> p q c", p=128)
            Lmod = constp.tile([128, CH, C], F16)
            Lot = tprocp.tile([128, 4, C], F32)
            Lev = tprocp.tile([128, 4, C], F32)
            nc.sync.dma_start(Lot[:], Lodd)
            nc.scalar.dma_start(Lev[:], Leven)

            # ---- Lmod: [+-(L_even - L_odd) | +-L_odd] in fp16 ----
            Ldif = tprocp.tile([128, 4, C], F32)
            nc.vector.scalar_tensor_tensor(
                Ldif[:], Lot[:], -1.0, Lev[:], op0=OP.mult, op1=OP.add
            )
            nc.scalar.activation(Lmod[:, 0:4, :], Ldif[:], AF.Copy, scale=sgn[:, 0:1])
            nc.scalar.activation(Lmod[:, 4:8, :], Lot[:], AF.Copy, scale=sgn[:, 0:1])

            # ---- main pipeline: per estimator-half on full-width tiles ----
            s_sb = bigp.tile([128, CH, ENP], I16)
            vT_full = bigp.tile([128, CH, BC], F16)
            y_ps = psumy.tile([128, BC], F32, tag="yT")
            for h in range(2):
                # xsum = hi + lo (fp32); s = rint((t - 0.5) - xsum), per
                # quarter so compute starts as soon as each gather lands
                for kk in range(2):
                    k = 2 * h + kk
                    xsum = accp.tile(
                        [128, CH, EQ], F32, tag=f"xsum{k % 2}",
                        name=f"xsum{k % 2}",
                    )
                    nc.vector.tensor_tensor(
                        xsum[:], xg[k][:, 0:8, :], xg[k][:, 8:16, :], op=OP.add
                    )
                    tb = t_bc[:, k * EQ : (k + 1) * EQ].unsqueeze(
                        1
                    ).broadcast_to([128, CH, EQ])
                    sq = s_sb[:, :, k * EQ : (k + 1) * EQ]
                    if SCAST == "dve":
                        nc.vector.tensor_tensor(sq, tb, xsum[:], op=OP.subtract)
                    else:
                        u = accp.tile(
                            [128, CH, EQ], F32, tag=f"u{k % 2}",
                            name=f"u{k % 2}",
                        )
                        nc.vector.tensor_tensor(u[:], tb, xsum[:], op=OP.subtract)
                        nc.scalar.activation(sq, u[:], AF.Copy)

                # tree per 4-chunk group so transposes overlap the tree;
                # s4 [128, 4, EHF, NNP]. Slots are host-permuted
                # (bit-reversed within each level) so every level is
                # written plane-major (all c0 children, then all c1) with
                # fully contiguous 2-byte operands.
                for g in range(2):
                    c0, c1 = 4 * g, 4 * g + 4
                    s4 = s_sb[:, c0:c1, h * EH : (h + 1) * EH].rearrange(
                        "p c (e n) -> p c e n", n=NNP
                    )
                    lvl1 = workp.tile([128, 4, EHF, 2], F16, tag="l1")
                    nc.scalar.activation(
                        lvl1[:, :, :, 0:1], s4[:, :, :, 0:1], AF.Copy
                    )
                    nc.scalar.activation(
                        lvl1[:, :, :, 1:2], s4[:, :, :, 0:1], AF.Copy, bias=-1.0
                    )
                    lvl = lvl1
                    v = workp.tile([128, 4, EH], F16, tag="v")
                    for j in range(2, DEPTH):  # levels 2..5
                        half = 2 ** (j - 1)
                        base = half - 1
                        if j < DEPTH - 1:
                            nxt = workp.tile(
                                [128, 4, EHF, 2 * half], F16, tag=f"l{j}"
                            )
                        else:
                            nxt = v[:, :, 256:512].rearrange(
                                "p c (e w) -> p c e w", w=2 * half
                            )
                        sj = s4[:, :, :, base : base + half]
                        nc.vector.tensor_tensor(
                            nxt[:, :, :, 0:half], sj, lvl[:], op=OP.mult
                        )
                        nc.vector.tensor_tensor(
                            nxt[:, :, :, half : 2 * half],
                            nxt[:, :, :, 0:half],
                            lvl[:],
                            op=OP.subtract,
                        )
                        if j < DEPTH - 1:
                            lvl = nxt
                    vA = v[:, :, 0:256].rearrange("p c (e m) -> p c e m", m=32)
                    vB = v[:, :, 256:512].rearrange("p c (e m) -> p c e m", m=32)
                    nc.vector.tensor_tensor(
                        vA, s4[:, :, :, 31:63], vB, op=OP.mult
                    )

                    # transposes: per b-chunk c, 4 el-blocks share a bank
                    for cc in range(4):
                        c = c0 + cc
                        tp = psumtp.tile([128, EH], F16, tag="tp")
                        for q in range(4):
                            nc.tensor.transpose(
                                tp[:, q * 128 : (q + 1) * 128],
                                v[:, cc, q * 128 : (q + 1) * 128],
                                idf[:],
                            )
                        nc.scalar.activation(
                            vT_full[
                                :, 4 * h : 4 * h + 4, c * 128 : (c + 1) * 128
                            ],
                            tp[:].rearrange("p (q z) -> p q z", z=128),
                            AF.Copy,
                        )

                # final matmul: Lmod el-chunks stationary, vT streams 512/bank
                for q in range(4):
                    # el-block (h, q): q<2 -> vA cols, else vB cols
                    lj = (h * 2 + q) if q < 2 else (4 + h * 2 + q - 2)
                    for w in range(2):
                        nc.tensor.matmul(
                            y_ps[:C, w * EH : (w + 1) * EH],
                            lhsT=Lmod[:, lj, :],
                            rhs=vT_full[:, 4 * h + q, w * EH : (w + 1) * EH],
                            start=(h == 0 and q == 0),
                            stop=(h == 1 and q == 3),
                        )

            # ---- tail: y^T -> y (PE transposes), then a vectorized
            # softmax per 4-chunk group with early output DMA ----
            ysb = constp.tile([128, BC], F32)
            nc.scalar.activation(ysb[:C, :], y_ps[:C, :], AF.Copy)
            yall = constp.tile([128, CH, C], F32)
            yout = constp.tile([128, CH, C], F32)
            for g in range(2):
                for cc in range(4):
                    c = 4 * g + cc
                    yt = psumsm.tile([128, C], F32, tag="yt")
                    nc.tensor.transpose(
                        yt[:, :], ysb[:C, c * 128 : (c + 1) * 128],
                        idf32[:C, :C],
                    )
                    nc.scalar.activation(yall[:, c, :], yt[:], AF.Copy)
                ysl = yall[:, 4 * g : 4 * g + 4, :]
                nm = workp.tile([128, 4, 1], F32, tag="nm")
                nc.vector.tensor_reduce(
                    nm[:], ysl, axis=AX.X, op=OP.max, negate=True
                )
                yd = workp.tile([128, 4, C], F32, tag="yd")
                nc.vector.tensor_tensor(
                    yd[:], ysl, nm[:].broadcast_to([128, 4, C]), op=OP.add
                )
                yexp = workp.tile([128, 4, C], F32, tag="yexp")
                nc.scalar.activation(yexp[:], yd[:], AF.Exp)
                ssum = workp.tile([128, 4, 1], F32, tag="ssum")
                nc.vector.tensor_reduce(ssum[:], yexp[:], axis=AX.X, op=OP.add)
                rec = workp.tile([128, 4, 1], F32, tag="rec")
                nc.vector.reciprocal(rec[:], ssum[:])
                nc.vector.tensor_tensor(
                    yout[:, 4 * g : 4 * g + 4, :], yexp[:],
                    rec[:].broadcast_to([128, 4, C]), op=OP.mult,
                )
                eng = nc.sync if g == 0 else nc.scalar
                eng.dma_start(
                    out_d[:, 4 * g : 4 * g + 4, :],
                    yout[:, 4 * g : 4 * g + 4, :],
                )

    nc.compile()
    return nc


_id_f16 = np.eye(128, dtype=np.float16)
_id_f32 = np.eye(128, dtype=np.float32)
_iotab_f32 = np.ascontiguousarray(
    np.broadcast_to(np.arange(D, dtype=np.float32), (128, D))
)
_p = np.arange(128)
_repq_f16 = (_p[:, None] % 16 == _p[None, :128] % 16).astype(np.float16)
_selm_f16 = np.zeros((128, 32), dtype=np.float16)
for _t in range(4):
    for _a in range(8):
        _selm_f16[_p // 16 == _a, 8 * _t + _a] = 1.0
_onesel_f32 = np.zeros((2, 256), dtype=np.float32)
for _t in range(2):
    _onesel_f32[_t, 128 * _t : 128 * (_t + 1)] = 1.0
_sgn_f32 = np.array(
    [(-1.0) ** bin(p % 32).count("1") for p in range(128)], dtype=np.float32
).reshape(128, 1)


def _bitrev(i, nbits):
    r = 0
    for _ in range(nbits):
        r = (r << 1) | (i & 1)
        i >>= 1
    return r


def _slot_perm():
    """slot n (within estimator) -> tree node index. Within each level
    the nodes are bit-reversed so the tree levels compute plane-major
    (all c0 children contiguously, then all c1)."""
    perm = np.zeros(NN, dtype=np.int64)
    for n in range(NN):
        j = (n + 1).bit_length()          # level 1..6
        base = (1 << (j - 1)) - 1
        perm[n] = base + _bitrev(n - base, j - 1)
    return perm


_PERM = _slot_perm()


def _make_tp(T):
    """Tp[r] for r = 8p + t: gather slot j = 512*(t//4) + 128*(t%4) + p;
    (e, n) = (j//64, j%64); node = PERM[n]; pad rows (n==63) are
    [1, 0, ...] so that tmax=1, argmax=0."""
    Tp = np.zeros((ENP, D), dtype=np.float32)
    r = np.arange(ENP)
    p, t = r // 8, r % 8
    j = 512 * (t // 4) + 128 * (t % 4) + p
    e, n = j // 64, j % 64
    real = n < NN
    Tp[real] = T[e[real], _PERM[n[real]]]
    Tp[~real, 0] = 1.0
    return Tp


def _make_lp(L):
    """Leaf-pair bit-reversal matching the plane-major tree: position i
    holds pair bitrev5(i)."""
    pairs = L.reshape(E, NL // 2, 2, C)
    rev = np.array([_bitrev(i, 5) for i in range(32)])
    return np.ascontiguousarray(pairs[:, rev].reshape(E, NL, C))


def make_in_maps(x, T, L):
    x = np.ascontiguousarray(x, dtype=np.float32)
    T = np.ascontiguousarray(T, dtype=np.float32)
    L = _make_lp(np.ascontiguousarray(L, dtype=np.float32))
    Tp = _make_tp(T)
    maps = []
    for i in range(NCORES):
        xs = x[i * BC : (i + 1) * BC]
        hi = xs.astype(np.float16)
        lo = (xs - hi.astype(np.float32)).astype(np.float16)
        xp = np.ascontiguousarray(
            np.concatenate([hi.T, lo.T], axis=1)
        )  # [D, 2*BC]
        maps.append({
            "xp": xp,
            "Tp": Tp,
            "L": L,
            "idf": _id_f16,
            "idf32": _id_f32,
            "iotab": _iotab_f32,
            "repq": _repq_f16,
            "selm": _selm_f16,
            "onesel": _onesel_f32,
            "sgn": _sgn_f32,
        })
    return maps


def run(x, T, L, trace=False, **kw):
    nc = build_program()
    res = run_bass_kernel_spmd(
        nc, make_in_maps(x, T, L), core_ids=list(range(NCORES)), trace=trace, **kw
    )
    out = np.concatenate(
        [
            res.results[i]["out"].transpose(1, 0, 2).reshape(BC, C)
            for i in range(NCORES)
        ],
        axis=0,
    )
    return out, res


def kernel(x, T, L):
    out, _ = run(x, T, L, trace=False)
    return out
